# revision 45
# baseline (speedup 1.0000x reference)
"""Tensor-parallel fused attention (QKV + RoPE + causal SDPA + out-proj) for
one TRN2 chip (8 NeuronCores), written in Bass/Tile.

Sharding: each core owns H/8 = 2 heads through QKV+RoPE+SDPA. The head
outputs are AllGathered (bf16, per batch+q-chunk) and the output projection
is sharded by OUTPUT columns (each core computes out[:, c*256:(c+1)*256]),
so the only collective is a cheap AllGather instead of an AllReduce. The
host assembles the full output by concatenating the 8 column slices.

Device compute is bf16 on the TensorEngine with fp32 PSUM accumulation;
softmax runs without max-subtraction (max |logit| ~ 5.8 for these inputs,
far from fp32/exp overflow). The attention is computed in the
"scores-transposed" orientation S^T[k, q] so no probability transpose is
needed for the A@V matmul: out^T[d, q] = V^T @ P^T with V in [k, d] layout
(one PE transpose of v per 128-token tile) and P^T read straight from SBUF.
The softmax denominator is accumulated on DVE and collapsed across
partitions with an all-ones matmul (which also broadcasts it to all 128
partitions for free).
"""

import os
import sys
import numpy as np

for _p in ("/opt/trn_rl_repo",):
    if _p not in sys.path:
        sys.path.insert(0, _p)

import ml_dtypes

import concourse.bass as bass
import concourse.mybir as mybir
import concourse.tile as tile
from concourse import bacc
from concourse.bass_utils import run_bass_kernel_spmd
from concourse.masks import make_identity
from concourse.tile_rust import add_dep_helper

BF16 = mybir.dt.bfloat16
F32 = mybir.dt.float32
P = 128          # head_dim == SBUF partitions
CH = 512         # token chunk (matmul moving N)

# full-size problem constants
B_FULL, T_FULL, D_FULL = 4, 2048, 2048
H_FULL = 16
N_CORES = 8


def build_nc(B, T, D, H, n_cores):
    """Build the per-core SPMD Bass graph. Returns compiled Bacc."""
    HPC = H // n_cores            # heads per core
    KT = D // P                   # k-tiles of the QKV contraction
    NQC = T // CH                 # q-chunks per batch
    NKT = T // P                  # k-tiles per batch (attention)
    TOK = B * T
    NOUT = D // n_cores           # out-proj columns per core
    HT = H                        # f-tiles (128 rows each) in out-proj
    SM_SCALE = 1.0 / float(np.sqrt(P))
    TPC = CH // P                 # 128-token tiles per chunk

    nc = bacc.Bacc("TRN2", target_bir_lowering=False, debug=False,
                   num_devices=n_cores)

    xT = nc.dram_tensor("xT", [D, TOK], BF16, kind="ExternalInput")
    wqkv = nc.dram_tensor("wqkv", [D, 3 * HPC * P], BF16, kind="ExternalInput")
    wout = nc.dram_tensor("wout", [H * P, NOUT], BF16, kind="ExternalInput")
    ropec = nc.dram_tensor("ropec", [P, T], BF16, kind="ExternalInput")
    ropes = nc.dram_tensor("ropes", [P, T], BF16, kind="ExternalInput")
    out = nc.dram_tensor("out", [TOK, NOUT], F32, kind="ExternalOutput")

    cc_in = [nc.dram_tensor(f"ccin{b}", [NQC, HPC * P, CH], BF16)
             for b in range(B)]
    cc_out = [nc.dram_tensor(f"ccout{b}", [NQC, H * P, CH], BF16,
                             addr_space="Shared") for b in range(B)]

    xT_r = xT.ap().rearrange("(kt p) n -> p kt n", p=P)
    wqkv_r = wqkv.ap().rearrange("(kt p) f -> p kt f", p=P)
    wout_r = wout.ap().rearrange("(ft p) n -> p ft n", p=P)

    with tile.TileContext(nc) as tc:
        from contextlib import ExitStack
        with ExitStack() as ctx:
            consts = ctx.enter_context(tc.tile_pool(name="consts", bufs=1))
            px = ctx.enter_context(tc.tile_pool(name="px", bufs=2))
            pqkv = ctx.enter_context(tc.tile_pool(name="pqkv", bufs=1))
            prope = ctx.enter_context(tc.tile_pool(name="prope", bufs=2))
            pexp = ctx.enter_context(tc.tile_pool(name="pexp", bufs=1))
            pden = ctx.enter_context(tc.tile_pool(name="pden", bufs=1))
            pv = ctx.enter_context(tc.tile_pool(name="pv", bufs=1))
            pao = ctx.enter_context(tc.tile_pool(name="pao", bufs=3))
            pop = ctx.enter_context(tc.tile_pool(name="pop", bufs=4))
            poo = ctx.enter_context(tc.tile_pool(name="poo", bufs=3))

            pp_qkv = ctx.enter_context(
                tc.tile_pool(name="pp_qkv", bufs=2, space="PSUM"))
            pp_sc = ctx.enter_context(
                tc.tile_pool(name="pp_sc", bufs=2, space="PSUM"))
            pp_av = ctx.enter_context(
                tc.tile_pool(name="pp_av", bufs=2, space="PSUM"))
            pp_tr = ctx.enter_context(
                tc.tile_pool(name="pp_tr", bufs=1, space="PSUM"))
            pp_op = ctx.enter_context(
                tc.tile_pool(name="pp_op", bufs=1, space="PSUM"))

            # --- resident constants ---
            wq_sb = consts.tile([P, KT, 3 * HPC * P], BF16)
            FH = 3 * HPC * P // 2
            nc.sync.dma_start(out=wq_sb[:, :, 0:FH], in_=wqkv_r[:, :, 0:FH])
            nc.sync.dma_start(out=wq_sb[:, :, FH:], in_=wqkv_r[:, :, FH:])
            wo_sb = consts.tile([P, HT, NOUT], BF16)
            nc.sync.dma_start(out=wo_sb[:], in_=wout_r)
            rc_sb = consts.tile([P, T], BF16)
            nc.sync.dma_start(out=rc_sb[:], in_=ropec.ap())
            rs_sb = consts.tile([P, T], BF16)
            nc.sync.dma_start(out=rs_sb[:], in_=ropes.ap())
            ones_sb = consts.tile([P, P], BF16)
            nc.vector.memset(ones_sb[:], 1.0)
            ident = consts.tile([P, P], BF16)
            make_identity(nc, ident[:])

            # tiny dummy AllGather to absorb the ~25us first-collective
            # warmup while QKV(0) computes
            warm_in = nc.dram_tensor("warm_in", [P, 16], BF16)
            warm_out = nc.dram_tensor("warm_out", [P * n_cores, 16], BF16,
                                      addr_space="Shared")
            warm_sb = consts.tile([P, 16], BF16)
            nc.vector.memset(warm_sb[:], 0.0)
            nc.sync.dma_start(out=warm_in.ap(), in_=warm_sb[:])
            nc.gpsimd.collective_compute(
                "AllGather", mybir.AluOpType.bypass,
                replica_groups=[list(range(n_cores))],
                ins=[warm_in.ap().opt()], outs=[warm_out.ap().opt()])

            def emit_outproj(b, qcs, order_after=None, halves=(0, 1)):
                for cc in qcs:
                    for half in halves:
                        hw = TPC // 2
                        a_sb = pop.tile([P, HT, hw * P], BF16, tag="opin")
                        src = cc_out[b].ap()[cc].rearrange(
                            "(ft p) t -> p ft t", p=P)
                        nc.sync.dma_start(
                            out=a_sb[:],
                            in_=src[:, :, half * hw * P:(half + 1) * hw * P])
                        for tt in range(hw):
                            po = pp_op.tile([P, NOUT], F32, tag="op")
                            for ft in range(HT):
                                mm = nc.tensor.matmul(
                                    po[:],
                                    a_sb[:, ft, tt * P:(tt + 1) * P],
                                    wo_sb[:, ft, :],
                                    start=(ft == 0), stop=(ft == HT - 1))
                                if order_after is not None:
                                    # keep these matmuls AFTER the newer
                                    # attention work in the PE stream: the
                                    # scheduler's cost model under-prices the
                                    # AllGather and would otherwise stall PE
                                    add_dep_helper(
                                        mm.ins, order_after.ins, sync=False,
                                        reason="outproj after attn PE order")
                                    order_after = None
                            oo = poo.tile([P, NOUT], F32, tag="oo")
                            nc.scalar.copy(oo[:], po[:])
                            r0 = b * T + cc * CH + (half * hw + tt) * P
                            nc.sync.dma_start(out=out.ap()[r0:r0 + P, :],
                                              in_=oo[:])

            def emit_qkv_chunk(b, cc, q_sb, k_sb, v_sb):
                pos0 = cc * CH
                tok0 = b * T + pos0
                x_sb = px.tile([P, KT, CH], BF16, tag="x", name="x")
                nc.sync.dma_start(out=x_sb[:],
                                  in_=xT_r[:, :, tok0:tok0 + CH])
                for fi in range(3 * HPC):
                    ps = pp_qkv.tile([P, CH], F32, tag="qkv", name="qkvps")
                    for kt in range(KT):
                        nc.tensor.matmul(
                            ps[:],
                            wq_sb[:, kt, fi * P:(fi + 1) * P],
                            x_sb[:, kt, :],
                            start=(kt == 0), stop=(kt == KT - 1))
                    if fi < 2 * HPC:  # q or k head: apply rope
                        h = fi % HPC
                        dst = (q_sb if fi < HPC else k_sb)
                        raw = prope.tile([P, CH], BF16, tag="raw", name="raw")
                        nc.scalar.copy(raw[:], ps[:])
                        sw = prope.tile([P, CH], BF16, tag="sw", name="sw")
                        half = P // 2
                        nc.sync.dma_start(out=sw[0:half, :],
                                          in_=raw[half:P, :])
                        nc.sync.dma_start(out=sw[half:P, :],
                                          in_=raw[0:half, :])
                        t1 = prope.tile([P, CH], F32, tag="t1", name="t1")
                        t2 = prope.tile([P, CH], F32, tag="t2", name="t2")
                        nc.vector.tensor_tensor(
                            t1[:], raw[:], rc_sb[:, pos0:pos0 + CH],
                            mybir.AluOpType.mult)
                        nc.vector.tensor_tensor(
                            t2[:], sw[:], rs_sb[:, pos0:pos0 + CH],
                            mybir.AluOpType.mult)
                        nc.vector.tensor_tensor(
                            dst[:, h, pos0:pos0 + CH], t1[:], t2[:],
                            mybir.AluOpType.add)
                    else:  # v head: copy + transpose into [tok, d] tiles
                        h = fi - 2 * HPC
                        vtc = prope.tile([P, CH], BF16, tag="vtc", name="vtc")
                        nc.scalar.copy(vtc[:], ps[:])
                        for tt in range(TPC):
                            kt_g = cc * TPC + tt
                            pst = pp_tr.tile([P, P], BF16, tag="tr",
                                             name="pst")
                            nc.tensor.transpose(
                                pst[:], vtc[:, tt * P:(tt + 1) * P],
                                ident[:])
                            nc.scalar.copy(v_sb[h][:, kt_g, :], pst[:])

            def emit_attn_chunk(b, qc, q_sb, k_sb, v_sb):
                """One attention q-chunk (both heads) + its AllGather.
                Returns the last AV matmul for PE-order pinning."""
                nkt = (qc + 1) * CH // P
                q0 = qc * CH
                diag0 = qc * CH // P
                expt = [None] * HPC
                den = [None] * HPC
                for h in range(HPC):
                    expt[h] = pexp.tile([P, NKT, CH], BF16,
                                        tag=f"e{h}", name=f"e{h}")
                    den[h] = pden.tile([P, CH], F32,
                                       tag=f"den{h}", name=f"den{h}")
                    for kt in range(nkt):
                        # columns qq < (kt-diag0)*P of a diagonal tile are
                        # fully masked: restrict all work to qq >= col0
                        col0 = (kt - diag0) * P if kt >= diag0 else 0
                        ncol = CH - col0
                        sc = pp_sc.tile([P, CH], F32, tag="sc", name="sc")
                        nc.tensor.matmul(
                            sc[:, col0:CH],
                            k_sb[:, h, kt * P:(kt + 1) * P],
                            q_sb[:, h, q0 + col0:q0 + CH],
                            start=True, stop=True)
                        es = expt[h][:, kt, col0:CH]
                        nc.scalar.activation(
                            es, sc[:, col0:CH],
                            mybir.ActivationFunctionType.Exp,
                            scale=SM_SCALE)
                        if kt >= diag0:
                            # causal within the restricted block: keep
                            # lower triangle (i >= kk)
                            nc.gpsimd.affine_select(
                                out=es, in_=es,
                                compare_op=mybir.AluOpType.is_ge,
                                fill=0.0, base=0,
                                channel_multiplier=-1,
                                pattern=[[1, ncol]])
                        if kt == 0:
                            # kt=0 always covers the full width; initialize
                            # the accumulator by copy
                            nc.vector.tensor_copy(den[h][:], es)
                        else:
                            nc.vector.tensor_tensor(
                                den[h][:, col0:CH], den[h][:, col0:CH],
                                es, mybir.AluOpType.add)
                last_av = None
                for h in range(HPC):
                    av = pp_av.tile([P, CH], F32, tag="av", name="av")
                    for kt in range(nkt):
                        col0 = (kt - diag0) * P if kt >= diag0 else 0
                        last_av = nc.tensor.matmul(
                            av[:, col0:CH], v_sb[h][:, kt, :],
                            expt[h][:, kt, col0:CH],
                            start=(kt == 0), stop=(kt == nkt - 1))
                    den_bf = pden.tile([P, CH], BF16, tag=f"db{h}",
                                       name=f"db{h}")
                    nc.vector.tensor_copy(den_bf[:], den[h][:])
                    # dbc shares the (QKV-phase-only) transpose bank so
                    # it doesn't throttle the QK->exp psum ring
                    dbc = pp_tr.tile([P, CH], F32, tag="tr", name="dbc")
                    nc.tensor.matmul(dbc[:], ones_sb[:], den_bf[:],
                                     start=True, stop=True)
                    rec1 = pden.tile([1, CH], F32, tag=f"r1{h}",
                                     name=f"r1{h}")
                    nc.vector.reciprocal_approx_fast(rec1[:], dbc[0:1, :])
                    recb = pden.tile([P, CH], F32, tag=f"rb{h}",
                                     name=f"rb{h}")
                    nc.gpsimd.partition_broadcast(recb[:], rec1[:])
                    ao = pao.tile([P, CH], BF16, tag="ao", name="ao")
                    nc.vector.tensor_tensor(ao[:], av[:], recb[:],
                                            mybir.AluOpType.mult)
                    nc.sync.dma_start(
                        out=cc_in[b].ap()[qc, h * P:(h + 1) * P, :],
                        in_=ao[:])
                nc.gpsimd.collective_compute(
                    "AllGather", mybir.AluOpType.bypass,
                    replica_groups=[list(range(n_cores))],
                    ins=[cc_in[b].ap()[qc].opt()],
                    outs=[cc_out[b].ap()[qc].opt()])
                return last_av

            for b in range(B):
                q_sb = pqkv.tile([P, HPC, T], BF16, tag="q", name="q")
                k_sb = pqkv.tile([P, HPC, T], BF16, tag="k", name="k")
                v_sb = [pv.tile([P, NKT, P], BF16, tag=f"v{h}", name=f"v{h}")
                        for h in range(HPC)]
                if b < B - 1:
                    # phase-separated: all QKV chunks, then attention
                    # (interleaving QKV with attention measured slower:
                    # rope-write / attention-read WAR ping-pong on q_sb/k_sb)
                    for cc in range(NQC):
                        emit_qkv_chunk(b, cc, q_sb, k_sb, v_sb)
                    for qc in range(NQC):
                        last_av = emit_attn_chunk(b, qc, q_sb, k_sb, v_sb)
                        if b >= 1:
                            emit_outproj(b - 1, [qc], order_after=last_av)
                else:
                    # last batch: all QKV first, then attention chunks in
                    # DESCENDING size order so the tail ends on the smallest
                    # chunk (shortest AllGather exposure), draining this
                    # batch's out-proj one processed-chunk behind
                    for cc in range(NQC):
                        emit_qkv_chunk(b, cc, q_sb, k_sb, v_sb)
                    def keep_warm(n):
                        # dep-free matmuls that execute during guaranteed
                        # AllGather-wait windows, keeping the PE clock (HAM)
                        # warm for the out-proj that follows
                        wp = pp_tr.tile([P, P], F32, tag="tr", name="warmps")
                        for _ in range(n):
                            nc.tensor.matmul(wp[:], ident[:], ident[:],
                                             start=True, stop=True)
                    prev_qc = None
                    for qc in reversed(range(NQC)):
                        last_av = emit_attn_chunk(b, qc, q_sb, k_sb, v_sb)
                        emit_outproj(b - 1, [qc], order_after=last_av)
                        if prev_qc is not None:
                            if qc <= 1:
                                keep_warm(40)
                            emit_outproj(b, [prev_qc], order_after=last_av)
                        prev_qc = qc
                    keep_warm(40)
                    emit_outproj(B - 1, [prev_qc])

    nc.compile()
    return nc


def shard_inputs(x, rope_cos, rope_sin, W_qkv, W_out, n_cores):
    """Host-side prep: transpose x, build rope tables in [d, pos] layout with
    the rotation sign folded in, slice per-core weight shards, cast to bf16."""
    B, T, D = x.shape
    H = W_qkv.shape[1] // (3 * P)
    HPC = H // n_cores
    NOUT = W_out.shape[1] // n_cores
    bf = ml_dtypes.bfloat16

    xT = np.ascontiguousarray(x.reshape(B * T, D).T).astype(bf)
    cosT = np.ascontiguousarray(rope_cos.T).astype(bf)          # [P, T]
    sinT = rope_sin.T.copy()
    sinT[:P // 2] = -sinT[:P // 2]
    sinT = np.ascontiguousarray(sinT).astype(bf)

    Wq3 = W_qkv.reshape(D, 3, H, P)  # [D, qkv, head, d]
    in_maps = []
    for c in range(n_cores):
        heads = range(c * HPC, (c + 1) * HPC)
        cols = [Wq3[:, i, h, :] for i in range(3) for h in heads]
        wqkv_c = np.ascontiguousarray(
            np.concatenate(cols, axis=1)).astype(bf)            # [D, 3*HPC*P]
        wout_c = np.ascontiguousarray(
            W_out[:, c * NOUT:(c + 1) * NOUT]).astype(bf)
        in_maps.append({
            "xT": xT, "wqkv": wqkv_c, "wout": wout_c,
            "ropec": cosT, "ropes": sinT,
        })
    return in_maps


def assemble_output(results, B, T, D, n_cores):
    NOUT = D // n_cores
    out = np.empty((B * T, D), np.float32)
    for c in range(n_cores):
        out[:, c * NOUT:(c + 1) * NOUT] = results[c]["out"]
    return out.reshape(B, T, D)


_NC_CACHE = {}


def _get_nc(B, T, D, H, n_cores):
    key = (B, T, D, H, n_cores)
    if key not in _NC_CACHE:
        _NC_CACHE[key] = build_nc(B, T, D, H, n_cores)
    return _NC_CACHE[key]


def run(x, rope_cos, rope_sin, W_qkv, W_out, trace=False):
    B, T, D = x.shape
    H = W_qkv.shape[1] // (3 * P)
    n_cores = N_CORES
    nc = _get_nc(B, T, D, H, n_cores)
    in_maps = shard_inputs(np.asarray(x, np.float32),
                           np.asarray(rope_cos, np.float32),
                           np.asarray(rope_sin, np.float32),
                           np.asarray(W_qkv, np.float32),
                           np.asarray(W_out, np.float32), n_cores)
    res = run_bass_kernel_spmd(nc, in_maps, core_ids=list(range(n_cores)),
                               trace=trace)
    out = assemble_output(res.results, B, T, D, n_cores)
    return out, res


def kernel(x, rope_cos, rope_sin, W_qkv, W_out):
    out, _ = run(x, rope_cos, rope_sin, W_qkv, W_out, trace=False)
    return out


# revision 46
# speedup vs baseline: 1.0095x; 1.0095x over previous
"""Tensor-parallel fused attention (QKV + RoPE + causal SDPA + out-proj) for
one TRN2 chip (8 NeuronCores), written in Bass/Tile.

Sharding: each core owns H/8 = 2 heads through QKV+RoPE+SDPA. The head
outputs are AllGathered (bf16, per batch+q-chunk) and the output projection
is sharded by OUTPUT columns (each core computes out[:, c*256:(c+1)*256]),
so the only collective is a cheap AllGather instead of an AllReduce. The
host assembles the full output by concatenating the 8 column slices.

Device compute is bf16 on the TensorEngine with fp32 PSUM accumulation;
softmax runs without max-subtraction (max |logit| ~ 5.8 for these inputs,
far from fp32/exp overflow). The attention is computed in the
"scores-transposed" orientation S^T[k, q] so no probability transpose is
needed for the A@V matmul: out^T[d, q] = V^T @ P^T with V in [k, d] layout
(one PE transpose of v per 128-token tile) and P^T read straight from SBUF.
The softmax denominator is accumulated on DVE and collapsed across
partitions with an all-ones matmul (which also broadcasts it to all 128
partitions for free).
"""

import os
import sys
import numpy as np

for _p in ("/opt/trn_rl_repo",):
    if _p not in sys.path:
        sys.path.insert(0, _p)

import ml_dtypes

import concourse.bass as bass
import concourse.mybir as mybir
import concourse.tile as tile
from concourse import bacc
from concourse.bass_utils import run_bass_kernel_spmd
from concourse.masks import make_identity
from concourse.tile_rust import add_dep_helper

BF16 = mybir.dt.bfloat16
F32 = mybir.dt.float32
P = 128          # head_dim == SBUF partitions
CH = 512         # token chunk (matmul moving N)

# full-size problem constants
B_FULL, T_FULL, D_FULL = 4, 2048, 2048
H_FULL = 16
N_CORES = 8


def build_nc(B, T, D, H, n_cores):
    """Build the per-core SPMD Bass graph. Returns compiled Bacc."""
    HPC = H // n_cores            # heads per core
    KT = D // P                   # k-tiles of the QKV contraction
    NQC = T // CH                 # q-chunks per batch
    NKT = T // P                  # k-tiles per batch (attention)
    TOK = B * T
    NOUT = D // n_cores           # out-proj columns per core
    HT = H                        # f-tiles (128 rows each) in out-proj
    SM_SCALE = 1.0 / float(np.sqrt(P))
    TPC = CH // P                 # 128-token tiles per chunk

    nc = bacc.Bacc("TRN2", target_bir_lowering=False, debug=False,
                   num_devices=n_cores)

    xT = nc.dram_tensor("xT", [D, TOK], BF16, kind="ExternalInput")
    wqkv = nc.dram_tensor("wqkv", [D, 3 * HPC * P], BF16, kind="ExternalInput")
    wout = nc.dram_tensor("wout", [H * P, NOUT], BF16, kind="ExternalInput")
    ropec = nc.dram_tensor("ropec", [P, T], BF16, kind="ExternalInput")
    ropes = nc.dram_tensor("ropes", [P, T], BF16, kind="ExternalInput")
    out = nc.dram_tensor("out", [TOK, NOUT], F32, kind="ExternalOutput")

    cc_in = [nc.dram_tensor(f"ccin{b}", [NQC, HPC * P, CH], BF16)
             for b in range(B)]
    cc_out = [nc.dram_tensor(f"ccout{b}", [NQC, H * P, CH], BF16,
                             addr_space="Shared") for b in range(B)]

    xT_r = xT.ap().rearrange("(kt p) n -> p kt n", p=P)
    wqkv_r = wqkv.ap().rearrange("(kt p) f -> p kt f", p=P)
    wout_r = wout.ap().rearrange("(ft p) n -> p ft n", p=P)

    with tile.TileContext(nc) as tc:
        from contextlib import ExitStack
        with ExitStack() as ctx:
            consts = ctx.enter_context(tc.tile_pool(name="consts", bufs=1))
            px = ctx.enter_context(tc.tile_pool(name="px", bufs=2))
            pqkv = ctx.enter_context(tc.tile_pool(name="pqkv", bufs=1))
            prope = ctx.enter_context(tc.tile_pool(name="prope", bufs=2))
            pexp = ctx.enter_context(tc.tile_pool(name="pexp", bufs=1))
            pden = ctx.enter_context(tc.tile_pool(name="pden", bufs=1))
            pv = ctx.enter_context(tc.tile_pool(name="pv", bufs=1))
            pao = ctx.enter_context(tc.tile_pool(name="pao", bufs=3))
            pop = ctx.enter_context(tc.tile_pool(name="pop", bufs=4))
            poo = ctx.enter_context(tc.tile_pool(name="poo", bufs=3))

            pp_qkv = ctx.enter_context(
                tc.tile_pool(name="pp_qkv", bufs=2, space="PSUM"))
            pp_sc = ctx.enter_context(
                tc.tile_pool(name="pp_sc", bufs=2, space="PSUM"))
            pp_av = ctx.enter_context(
                tc.tile_pool(name="pp_av", bufs=2, space="PSUM"))
            pp_tr = ctx.enter_context(
                tc.tile_pool(name="pp_tr", bufs=1, space="PSUM"))
            pp_op = ctx.enter_context(
                tc.tile_pool(name="pp_op", bufs=1, space="PSUM"))

            # --- resident constants ---
            wq_sb = consts.tile([P, KT, 3 * HPC * P], BF16)
            FH = 3 * HPC * P // 2
            nc.sync.dma_start(out=wq_sb[:, :, 0:FH], in_=wqkv_r[:, :, 0:FH])
            nc.sync.dma_start(out=wq_sb[:, :, FH:], in_=wqkv_r[:, :, FH:])
            wo_sb = consts.tile([P, HT, NOUT], BF16)
            nc.sync.dma_start(out=wo_sb[:], in_=wout_r)
            rc_sb = consts.tile([P, T], BF16)
            nc.sync.dma_start(out=rc_sb[:], in_=ropec.ap())
            rs_sb = consts.tile([P, T], BF16)
            nc.sync.dma_start(out=rs_sb[:], in_=ropes.ap())
            ones_sb = consts.tile([P, P], BF16)
            nc.vector.memset(ones_sb[:], 1.0)
            ident = consts.tile([P, P], BF16)
            make_identity(nc, ident[:])

            # tiny dummy AllGather to absorb the ~25us first-collective
            # warmup while QKV(0) computes
            warm_in = nc.dram_tensor("warm_in", [P, 16], BF16)
            warm_out = nc.dram_tensor("warm_out", [P * n_cores, 16], BF16,
                                      addr_space="Shared")
            warm_sb = consts.tile([P, 16], BF16)
            nc.vector.memset(warm_sb[:], 0.0)
            nc.sync.dma_start(out=warm_in.ap(), in_=warm_sb[:])
            nc.gpsimd.collective_compute(
                "AllGather", mybir.AluOpType.bypass,
                replica_groups=[list(range(n_cores))],
                ins=[warm_in.ap().opt()], outs=[warm_out.ap().opt()])

            def emit_outproj(b, qcs, order_after=None, halves=(0, 1)):
                for cc in qcs:
                    for half in halves:
                        hw = TPC // 2
                        a_sb = pop.tile([P, HT, hw * P], BF16, tag="opin")
                        src = cc_out[b].ap()[cc].rearrange(
                            "(ft p) t -> p ft t", p=P)
                        nc.sync.dma_start(
                            out=a_sb[:],
                            in_=src[:, :, half * hw * P:(half + 1) * hw * P])
                        for tt in range(hw):
                            po = pp_op.tile([P, NOUT], F32, tag="op")
                            for ft in range(HT):
                                mm = nc.tensor.matmul(
                                    po[:],
                                    a_sb[:, ft, tt * P:(tt + 1) * P],
                                    wo_sb[:, ft, :],
                                    start=(ft == 0), stop=(ft == HT - 1))
                                if order_after is not None:
                                    # keep these matmuls AFTER the newer
                                    # attention work in the PE stream: the
                                    # scheduler's cost model under-prices the
                                    # AllGather and would otherwise stall PE
                                    add_dep_helper(
                                        mm.ins, order_after.ins, sync=False,
                                        reason="outproj after attn PE order")
                                    order_after = None
                            oo = poo.tile([P, NOUT], F32, tag="oo")
                            nc.scalar.copy(oo[:], po[:])
                            r0 = b * T + cc * CH + (half * hw + tt) * P
                            nc.sync.dma_start(out=out.ap()[r0:r0 + P, :],
                                              in_=oo[:])

            def emit_qkv_chunk(b, cc, q_sb, k_sb, v_sb):
                pos0 = cc * CH
                tok0 = b * T + pos0
                x_sb = px.tile([P, KT, CH], BF16, tag="x", name="x")
                nc.sync.dma_start(out=x_sb[:],
                                  in_=xT_r[:, :, tok0:tok0 + CH])
                for fi in range(3 * HPC):
                    ps = pp_qkv.tile([P, CH], F32, tag="qkv", name="qkvps")
                    for kt in range(KT):
                        nc.tensor.matmul(
                            ps[:],
                            wq_sb[:, kt, fi * P:(fi + 1) * P],
                            x_sb[:, kt, :],
                            start=(kt == 0), stop=(kt == KT - 1))
                    if fi < 2 * HPC:  # q or k head: apply rope
                        h = fi % HPC
                        dst = (q_sb if fi < HPC else k_sb)
                        raw = prope.tile([P, CH], BF16, tag="raw", name="raw")
                        nc.scalar.copy(raw[:], ps[:])
                        sw = prope.tile([P, CH], BF16, tag="sw", name="sw")
                        half = P // 2
                        nc.sync.dma_start(out=sw[0:half, :],
                                          in_=raw[half:P, :])
                        nc.sync.dma_start(out=sw[half:P, :],
                                          in_=raw[0:half, :])
                        t1 = prope.tile([P, CH], F32, tag="t1", name="t1")
                        t2 = prope.tile([P, CH], F32, tag="t2", name="t2")
                        nc.vector.tensor_tensor(
                            t1[:], raw[:], rc_sb[:, pos0:pos0 + CH],
                            mybir.AluOpType.mult)
                        nc.vector.tensor_tensor(
                            t2[:], sw[:], rs_sb[:, pos0:pos0 + CH],
                            mybir.AluOpType.mult)
                        nc.vector.tensor_tensor(
                            dst[:, h, pos0:pos0 + CH], t1[:], t2[:],
                            mybir.AluOpType.add)
                    else:  # v head: copy + transpose into [tok, d] tiles
                        h = fi - 2 * HPC
                        vtc = prope.tile([P, CH], BF16, tag="vtc", name="vtc")
                        nc.scalar.copy(vtc[:], ps[:])
                        for tt in range(TPC):
                            kt_g = cc * TPC + tt
                            pst = pp_tr.tile([P, P], BF16, tag="tr",
                                             name="pst")
                            nc.tensor.transpose(
                                pst[:], vtc[:, tt * P:(tt + 1) * P],
                                ident[:])
                            nc.scalar.copy(v_sb[h][:, kt_g, :], pst[:])

            def emit_attn_chunk(b, qc, q_sb, k_sb, v_sb):
                """One attention q-chunk (both heads) + its AllGather.
                Returns the last AV matmul for PE-order pinning."""
                nkt = (qc + 1) * CH // P
                q0 = qc * CH
                diag0 = qc * CH // P
                expt = [None] * HPC
                den = [None] * HPC
                for h in range(HPC):
                    expt[h] = pexp.tile([P, NKT, CH], BF16,
                                        tag=f"e{h}", name=f"e{h}")
                    den[h] = pden.tile([P, CH], F32,
                                       tag=f"den{h}", name=f"den{h}")
                    for kt in range(nkt):
                        # columns qq < (kt-diag0)*P of a diagonal tile are
                        # fully masked: restrict all work to qq >= col0
                        col0 = (kt - diag0) * P if kt >= diag0 else 0
                        ncol = CH - col0
                        sc = pp_sc.tile([P, CH], F32, tag="sc", name="sc")
                        nc.tensor.matmul(
                            sc[:, col0:CH],
                            k_sb[:, h, kt * P:(kt + 1) * P],
                            q_sb[:, h, q0 + col0:q0 + CH],
                            start=True, stop=True)
                        es = expt[h][:, kt, col0:CH]
                        nc.scalar.activation(
                            es, sc[:, col0:CH],
                            mybir.ActivationFunctionType.Exp,
                            scale=SM_SCALE)
                        if kt >= diag0:
                            # causal within the restricted block: keep
                            # lower triangle (i >= kk)
                            nc.gpsimd.affine_select(
                                out=es, in_=es,
                                compare_op=mybir.AluOpType.is_ge,
                                fill=0.0, base=0,
                                channel_multiplier=-1,
                                pattern=[[1, ncol]])
                        if kt == 0:
                            # kt=0 always covers the full width; initialize
                            # the accumulator by copy
                            nc.vector.tensor_copy(den[h][:], es)
                        else:
                            nc.vector.tensor_tensor(
                                den[h][:, col0:CH], den[h][:, col0:CH],
                                es, mybir.AluOpType.add)
                last_av = None
                for h in range(HPC):
                    av = pp_av.tile([P, CH], F32, tag="av", name="av")
                    for kt in range(nkt):
                        col0 = (kt - diag0) * P if kt >= diag0 else 0
                        last_av = nc.tensor.matmul(
                            av[:, col0:CH], v_sb[h][:, kt, :],
                            expt[h][:, kt, col0:CH],
                            start=(kt == 0), stop=(kt == nkt - 1))
                    den_bf = pden.tile([P, CH], BF16, tag=f"db{h}",
                                       name=f"db{h}")
                    nc.vector.tensor_copy(den_bf[:], den[h][:])
                    # dbc shares the (QKV-phase-only) transpose bank so
                    # it doesn't throttle the QK->exp psum ring
                    dbc = pp_tr.tile([P, CH], F32, tag="tr", name="dbc")
                    nc.tensor.matmul(dbc[:], ones_sb[:], den_bf[:],
                                     start=True, stop=True)
                    rec1 = pden.tile([1, CH], F32, tag=f"r1{h}",
                                     name=f"r1{h}")
                    nc.vector.reciprocal_approx_fast(rec1[:], dbc[0:1, :])
                    recb = pden.tile([P, CH], F32, tag=f"rb{h}",
                                     name=f"rb{h}")
                    nc.gpsimd.partition_broadcast(recb[:], rec1[:])
                    ao = pao.tile([P, CH], BF16, tag="ao", name="ao")
                    nc.vector.tensor_tensor(ao[:], av[:], recb[:],
                                            mybir.AluOpType.mult)
                    nc.sync.dma_start(
                        out=cc_in[b].ap()[qc, h * P:(h + 1) * P, :],
                        in_=ao[:])
                nc.gpsimd.collective_compute(
                    "AllGather", mybir.AluOpType.bypass,
                    replica_groups=[list(range(n_cores))],
                    ins=[cc_in[b].ap()[qc].opt()],
                    outs=[cc_out[b].ap()[qc].opt()])
                return last_av

            for b in range(B):
                q_sb = pqkv.tile([P, HPC, T], BF16, tag="q", name="q")
                k_sb = pqkv.tile([P, HPC, T], BF16, tag="k", name="k")
                v_sb = [pv.tile([P, NKT, P], BF16, tag=f"v{h}", name=f"v{h}")
                        for h in range(HPC)]
                if b < B - 1:
                    # phase-separated: all QKV chunks, then attention
                    # (interleaving QKV with attention measured slower:
                    # rope-write / attention-read WAR ping-pong on q_sb/k_sb)
                    for cc in range(NQC):
                        emit_qkv_chunk(b, cc, q_sb, k_sb, v_sb)
                    for qc in range(NQC):
                        last_av = emit_attn_chunk(b, qc, q_sb, k_sb, v_sb)
                        if b >= 1:
                            emit_outproj(b - 1, [qc], order_after=last_av)
                else:
                    # last batch: all QKV first, then attention chunks in
                    # DESCENDING size order so the tail ends on the smallest
                    # chunk (shortest AllGather exposure), draining this
                    # batch's out-proj one processed-chunk behind
                    for cc in range(NQC):
                        emit_qkv_chunk(b, cc, q_sb, k_sb, v_sb)
                    prev_qc = None
                    for qc in reversed(range(NQC)):
                        last_av = emit_attn_chunk(b, qc, q_sb, k_sb, v_sb)
                        emit_outproj(b - 1, [qc], order_after=last_av)
                        if prev_qc is not None:
                            emit_outproj(b, [prev_qc], order_after=last_av)
                        prev_qc = qc
                    emit_outproj(B - 1, [prev_qc])

    nc.compile()
    return nc


def shard_inputs(x, rope_cos, rope_sin, W_qkv, W_out, n_cores):
    """Host-side prep: transpose x, build rope tables in [d, pos] layout with
    the rotation sign folded in, slice per-core weight shards, cast to bf16."""
    B, T, D = x.shape
    H = W_qkv.shape[1] // (3 * P)
    HPC = H // n_cores
    NOUT = W_out.shape[1] // n_cores
    bf = ml_dtypes.bfloat16

    xT = np.ascontiguousarray(x.reshape(B * T, D).T).astype(bf)
    cosT = np.ascontiguousarray(rope_cos.T).astype(bf)          # [P, T]
    sinT = rope_sin.T.copy()
    sinT[:P // 2] = -sinT[:P // 2]
    sinT = np.ascontiguousarray(sinT).astype(bf)

    Wq3 = W_qkv.reshape(D, 3, H, P)  # [D, qkv, head, d]
    in_maps = []
    for c in range(n_cores):
        heads = range(c * HPC, (c + 1) * HPC)
        cols = [Wq3[:, i, h, :] for i in range(3) for h in heads]
        wqkv_c = np.ascontiguousarray(
            np.concatenate(cols, axis=1)).astype(bf)            # [D, 3*HPC*P]
        wout_c = np.ascontiguousarray(
            W_out[:, c * NOUT:(c + 1) * NOUT]).astype(bf)
        in_maps.append({
            "xT": xT, "wqkv": wqkv_c, "wout": wout_c,
            "ropec": cosT, "ropes": sinT,
        })
    return in_maps


def assemble_output(results, B, T, D, n_cores):
    NOUT = D // n_cores
    out = np.empty((B * T, D), np.float32)
    for c in range(n_cores):
        out[:, c * NOUT:(c + 1) * NOUT] = results[c]["out"]
    return out.reshape(B, T, D)


_NC_CACHE = {}


def _get_nc(B, T, D, H, n_cores):
    key = (B, T, D, H, n_cores)
    if key not in _NC_CACHE:
        _NC_CACHE[key] = build_nc(B, T, D, H, n_cores)
    return _NC_CACHE[key]


def run(x, rope_cos, rope_sin, W_qkv, W_out, trace=False):
    B, T, D = x.shape
    H = W_qkv.shape[1] // (3 * P)
    n_cores = N_CORES
    nc = _get_nc(B, T, D, H, n_cores)
    in_maps = shard_inputs(np.asarray(x, np.float32),
                           np.asarray(rope_cos, np.float32),
                           np.asarray(rope_sin, np.float32),
                           np.asarray(W_qkv, np.float32),
                           np.asarray(W_out, np.float32), n_cores)
    res = run_bass_kernel_spmd(nc, in_maps, core_ids=list(range(n_cores)),
                               trace=trace)
    out = assemble_output(res.results, B, T, D, n_cores)
    return out, res


def kernel(x, rope_cos, rope_sin, W_qkv, W_out):
    out, _ = run(x, rope_cos, rope_sin, W_qkv, W_out, trace=False)
    return out
